# revision 1
# baseline (speedup 1.0000x reference)
"""DAriEL_Encoder_Cell_2 Trainium2 kernel (8-core SPMD, Bass/Tile).

Reformulation of the reference (verified to ~3e-7 max rel err vs oracle):
  - The reference re-runs the LM-LSTM from scratch on every prefix; since the
    LSTM is causal, we instead run it incrementally: one LSTM step per output
    step t (7 real steps; step 0 is fully masked PAD and z_0 == 0).
  - For t>=1 the arithmetic-coder bounds are reset to [0, 3] each step, so
    z_t = 1.5 * (cl + cu) where cl/cu are the exclusive/inclusive softmax
    cumsums at the current token s: with h in (-1,1) no max-subtraction is
    needed and z_t = 1.5 * (num_lt + num_le) / denom with
    num_lt = sum_{v<s} exp(h_v), num_le = sum_{v<=s} exp(h_v),
    denom = sum_v exp(h_v).
  - x@Wi + b is folded into one table EWi = E@Wi + b (weight-only transform);
    per step the token's row is gathered by indirect DMA and added to the
    gate PSUM with a tiny identity matmul.

Sharding: 8-way tensor parallel over the 4*2048 gate columns (256 hidden
units per core).  Each core keeps its Wh shard [2048, 1024] resident in SBUF
and computes gates for its slice; h slices (transposed) plus per-core softmax
partial stats are exchanged with one AllGather per step.
"""

import numpy as np

B, T, V, EMB, LAT = 32, 8, 2048, 256, 128
H = V                      # LSTM units == vocab size
NCORES = 8
HS = H // NCORES           # 256 hidden units per core
GS = 4 * HS                # 1024 gate columns per core
KC = H // 128              # 16 contraction chunks
AGR = 256                  # AG rows per rank: h^T slice only
SIZE_LAT = 3.0

_CACHE = {}


def _build_program():
    import concourse.bacc as bacc
    import concourse.bass as bass
    import concourse.mybir as mybir
    import concourse.tile as tile

    f32 = mybir.dt.float32
    f32r = mybir.dt.float32r
    i32 = mybir.dt.int32
    Alu = mybir.AluOpType
    Act = mybir.ActivationFunctionType

    nc = bacc.Bacc(
        "TRN2",
        target_bir_lowering=False,
        debug=False,
        num_devices=NCORES,
    )

    wh = nc.dram_tensor("wh", [H, GS], f32r, kind="ExternalInput")
    ewi = nc.dram_tensor("ewi", [V, GS], f32r, kind="ExternalInput")
    tok = nc.dram_tensor("tok", [B, T], i32, kind="ExternalInput")
    iota = nc.dram_tensor("iota", [B, HS], f32, kind="ExternalInput")
    sel3 = nc.dram_tensor("sel3", [32, 2], f32, kind="ExternalInput")
    idn = nc.dram_tensor("idn", [B, B], f32, kind="ExternalInput")
    idnr = nc.dram_tensor("idnr", [B, B], f32r, kind="ExternalInput")
    zout = nc.dram_tensor("z", [B, T, LAT], f32, kind="ExternalOutput")

    # internal DRAM bounce buffers for the per-step AllGather
    agin2 = [
        nc.dram_tensor(f"agin2_{t}", [4, B], f32, kind="Internal")
        for t in range(1, T)
    ]
    agout2 = [
        nc.dram_tensor(f"agout2_{t}", [4 * NCORES, B], f32, kind="Internal",
                       addr_space="Shared")
        for t in range(1, T)
    ]
    agin = [
        nc.dram_tensor(f"agin{t}", [AGR, B], f32, kind="Internal")
        for t in range(1, T)
    ]
    agout = [
        nc.dram_tensor(
            f"agout{t}", [AGR * NCORES, B], f32, kind="Internal",
            addr_space="Shared",
        )
        for t in range(1, T)
    ]

    with tile.TileContext(nc) as tc:
        with (
            tc.tile_pool(name="const", bufs=1) as constp,
            tc.tile_pool(name="whp", bufs=1) as whp,
            tc.tile_pool(name="ewip", bufs=1) as ewip,
            tc.tile_pool(name="hts", bufs=2) as htsp,
            tc.tile_pool(name="work", bufs=2) as workp,
            tc.tile_pool(name="state", bufs=2) as statep,
            tc.tile_pool(name="gpsum", bufs=2, space="PSUM") as gpsump,
            tc.tile_pool(name="tpsum", bufs=1, space="PSUM") as tpsump,
            tc.tile_pool(name="spsum", bufs=1, space="PSUM") as spsump,
        ):
            # ---------------- prologue: constants + weights ----------------
            tok_sb = constp.tile([B, T], i32, tag="tok")
            nc.sync.dma_start(tok_sb[:], tok[:])
            tokf = constp.tile([B, T], f32, tag="tokf")
            nc.vector.tensor_copy(tokf[:], tok_sb[:])

            iota_sb = constp.tile([B, HS], f32, tag="iota")
            nc.sync.dma_start(iota_sb[:], iota[:])
            sel3_sb = constp.tile([32, 2], f32, tag="sel3")
            nc.sync.dma_start(sel3_sb[:], sel3[:])
            idn_sb = constp.tile([B, B], f32, tag="idn")
            nc.sync.dma_start(idn_sb[:], idn[:])
            idnr_sb = constp.tile([B, B], f32r, tag="idnr")
            nc.sync.dma_start(idnr_sb[:], idnr[:])

            wh_sb = []
            for j in range(KC):
                wt = whp.tile([128, GS], f32r, tag=f"wh{j}", name=f"wh{j}")
                nc.sync.dma_start(wt[:], wh[j * 128:(j + 1) * 128, :])
                wh_sb.append(wt)

            # gather the EWi rows for every step's input token up front
            ewi_sb = []
            for t in range(1, T):
                et = ewip.tile([B, GS], f32r, tag=f"ewi{t}", name=f"ewi{t}")
                nc.gpsimd.indirect_dma_start(
                    out=et[:],
                    out_offset=None,
                    in_=ewi[:],
                    in_offset=bass.IndirectOffsetOnAxis(
                        ap=tok_sb[:, t - 1:t], axis=0
                    ),
                )
                ewi_sb.append(et)

            zfull = constp.tile([B, T * LAT], f32, tag="zfull")
            nc.vector.memset(zfull[:, 0:LAT], 0.0)

            h_prev = None
            c_prev = None
            hts = None

            for t in range(1, T):
                # ---------------- gate GEMM ----------------
                psA = gpsump.tile([B, 512], f32, tag="psA", name=f"psA{t}")
                psB = gpsump.tile([B, 512], f32, tag="psB", name=f"psB{t}")
                ew = ewi_sb[t - 1]
                for half, ps in ((0, psA), (1, psB)):
                    n0 = half * 512
                    if t == 1:
                        nc.tensor.matmul(
                            ps[:], idnr_sb[:], ew[:, n0:n0 + 512],
                            start=True, stop=True,
                        )
                    else:
                        for j in range(KC):
                            nc.tensor.matmul(
                                ps[:],
                                hts[:, 32 * j:32 * j + 32],
                                wh_sb[j][:, n0:n0 + 512],
                                start=(j == 0), stop=False,
                            )
                        nc.tensor.matmul(
                            ps[:], idnr_sb[:], ew[:, n0:n0 + 512],
                            start=False, stop=True,
                        )

                # ---------------- nonlinearity + state update --------------
                # shard column order is [i | f | o | g].  ScalarE only ever
                # runs Sigmoid (tanh(x) = 2*sigmoid(2x)-1, exp via
                # s/(1-s)) so the activation table is never reloaded.
                sigA = workp.tile([B, 512], f32, tag="sigA", name=f"sigA{t}")
                nc.scalar.activation(sigA[:], psA[:], Act.Sigmoid)
                sigO = workp.tile([B, HS], f32, tag="sigO", name=f"sigO{t}")
                nc.scalar.activation(sigO[:], psB[:, 0:HS], Act.Sigmoid)
                sigG = workp.tile([B, HS], f32, tag="sigG", name=f"sigG{t}")
                nc.scalar.activation(
                    sigG[:], psB[:, HS:2 * HS], Act.Sigmoid, scale=2.0
                )

                m_t = workp.tile([B, 1], f32, tag="mt", name=f"mt{t}")
                nc.vector.tensor_scalar(
                    m_t[:], tokf[:, t - 1:t], 0.0, None, Alu.is_gt
                )

                # ig = i * tanh(g) = 2*(i*sigG) - i
                ig0 = workp.tile([B, HS], f32, tag="ig0", name=f"ig0{t}")
                nc.vector.tensor_mul(ig0[:], sigA[:, 0:HS], sigG[:])
                ig = workp.tile([B, HS], f32, tag="ig", name=f"ig{t}")
                nc.vector.scalar_tensor_tensor(
                    out=ig[:], in0=ig0[:], scalar=2.0, in1=sigA[:, 0:HS],
                    op0=Alu.mult, op1=Alu.subtract,
                )

                c_t = statep.tile([B, HS], f32, tag="c", name=f"c{t}")
                h_t = statep.tile([B, HS], f32, tag="h", name=f"h{t}")
                if t == 1:
                    # c_prev = h_prev = 0
                    nc.vector.tensor_scalar(
                        c_t[:], ig[:], m_t[:, 0:1], None, Alu.mult
                    )
                else:
                    fc = workp.tile([B, HS], f32, tag="fc", name=f"fc{t}")
                    nc.vector.tensor_mul(fc[:], sigA[:, HS:2 * HS], c_prev[:])
                    cn = workp.tile([B, HS], f32, tag="cn", name=f"cn{t}")
                    nc.vector.tensor_add(cn[:], ig[:], fc[:])
                    dc = workp.tile([B, HS], f32, tag="dc", name=f"dc{t}")
                    nc.vector.tensor_sub(dc[:], cn[:], c_prev[:])
                    # c_t = m * dc + c_prev
                    nc.vector.scalar_tensor_tensor(
                        out=c_t[:], in0=dc[:], scalar=m_t[:, 0:1],
                        in1=c_prev[:], op0=Alu.mult, op1=Alu.add,
                    )

                sigC = workp.tile([B, HS], f32, tag="sigC", name=f"sigC{t}")
                nc.scalar.activation(sigC[:], c_t[:], Act.Sigmoid, scale=2.0)
                # hn = o * tanh(c) = 2*(o*sigC) - o
                hn0 = workp.tile([B, HS], f32, tag="hn0", name=f"hn0{t}")
                nc.vector.tensor_mul(hn0[:], sigO[:], sigC[:])
                hn = workp.tile([B, HS], f32, tag="hn", name=f"hn{t}")
                nc.vector.scalar_tensor_tensor(
                    out=hn[:], in0=hn0[:], scalar=2.0, in1=sigO[:],
                    op0=Alu.mult, op1=Alu.subtract,
                )
                if t == 1:
                    nc.vector.tensor_scalar(
                        h_t[:], hn[:], m_t[:, 0:1], None, Alu.mult
                    )
                else:
                    dh = workp.tile([B, HS], f32, tag="dh", name=f"dh{t}")
                    nc.vector.tensor_sub(dh[:], hn[:], h_prev[:])
                    nc.vector.scalar_tensor_tensor(
                        out=h_t[:], in0=dh[:], scalar=m_t[:, 0:1],
                        in1=h_prev[:], op0=Alu.mult, op1=Alu.add,
                    )
                c_prev, h_prev = c_t, h_t

                # ------------- local softmax partial stats -----------------
                # exp(h) = s/(1-s) with s = sigmoid(h)
                stk = workp.tile([B, 4], f32, tag="stk", name=f"stk{t}")
                nc.vector.memset(stk[:, 3:4], 0.0)
                sigH = workp.tile([B, HS], f32, tag="sigH", name=f"sigH{t}")
                nc.scalar.activation(sigH[:], h_t[:], Act.Sigmoid)
                omh = workp.tile([B, HS], f32, tag="omh", name=f"omh{t}")
                nc.vector.tensor_scalar(
                    omh[:], sigH[:], -1.0, 1.0, Alu.mult, Alu.add
                )
                rch = workp.tile([B, HS], f32, tag="rch", name=f"rch{t}")
                nc.vector.reciprocal(rch[:], omh[:])
                expk = workp.tile([B, HS], f32, tag="expk", name=f"expk{t}")
                nc.vector.scalar_tensor_tensor(
                    out=expk[:], in0=sigH[:], scalar=1.0, in1=rch[:],
                    op0=Alu.mult, op1=Alu.mult, accum_out=stk[:, 0:1],
                )
                junk = workp.tile([B, HS], f32, tag="junk", name=f"junk{t}")
                # num_lt = sum((iota < s) * exp)
                nc.vector.scalar_tensor_tensor(
                    out=junk[:], in0=iota_sb[:], scalar=tokf[:, t:t + 1],
                    in1=expk[:], op0=Alu.is_lt, op1=Alu.mult,
                    accum_out=stk[:, 1:2],
                )
                nc.vector.scalar_tensor_tensor(
                    out=junk[:], in0=iota_sb[:], scalar=tokf[:, t:t + 1],
                    in1=expk[:], op0=Alu.is_le, op1=Alu.mult,
                    accum_out=stk[:, 2:3],
                )

                # ------------- transpose h + stats, ship to AG -------------
                tp = tpsump.tile([128, 96], f32, tag="tp", name=f"tp{t}")
                nc.tensor.transpose(tp[:, 0:32], h_t[:, 0:128], idn_sb[:])
                nc.tensor.transpose(tp[:, 32:64], h_t[:, 128:256], idn_sb[:])
                nc.tensor.transpose(tp[0:4, 64:96], stk[:], idn_sb[:])
                tps = workp.tile([128, 96], f32, tag="tps", name=f"tps{t}")
                nc.vector.tensor_copy(tps[:, 0:64], tp[:, 0:64])
                nc.vector.tensor_copy(tps[0:4, 64:96], tp[0:4, 64:96])

                ag_i = agin[t - 1].ap()
                nc.sync.dma_start(
                    ag_i[:].rearrange("(c p) b -> p c b", c=2),
                    tps[:, 0:64].rearrange("p (c b) -> p c b", c=2),
                )
                nc.gpsimd.collective_compute(
                    "AllGather",
                    Alu.bypass,
                    replica_groups=[list(range(NCORES))],
                    ins=[agin[t - 1].ap()],
                    outs=[agout[t - 1].ap()],
                )
                # stats ride a second, smaller AG that lags the critical path
                nc.sync.dma_start(agin2[t - 1].ap(), tps[0:4, 64:96])
                nc.gpsimd.collective_compute(
                    "AllGather",
                    Alu.bypass,
                    replica_groups=[list(range(NCORES))],
                    ins=[agin2[t - 1].ap()],
                    outs=[agout2[t - 1].ap()],
                )

                ag_o = agout[t - 1].ap()
                stats_sb = workp.tile([4 * NCORES, B], f32, tag="stats",
                                      name=f"st{t}")
                nc.sync.dma_start(stats_sb[:], agout2[t - 1].ap())
                if t < T - 1:
                    # gathered h^T -> SBUF [128, 512]; 8 DMAs for parallelism
                    hts = htsp.tile([128, KC * 32], f32r, tag="hts",
                                    name=f"hts{t}")
                    for q in range(NCORES):
                        nc.sync.dma_start(
                            hts[:, q * 64:(q + 1) * 64].rearrange(
                                "p (c b) -> p c b", c=2
                            ),
                            ag_o[AGR * q:AGR * q + 256, :]
                            .bitcast(f32r)
                            .rearrange("(c p) b -> p c b", c=2),
                        )

                # ------------- finish z_t ----------------------------------
                # ps3[0, 0:32]  = 1.5*(num_lt + num_le)   (sel2 col 0)
                # ps3[0, 32:64] = denom                   (sel2 col 1)
                ps3 = spsump.tile([1, 2 * B], f32, tag="ps3", name=f"ps3{t}")
                nc.tensor.matmul(
                    ps3[0:1, 0:B], sel3_sb[:, 0:1], stats_sb[:],
                    start=True, stop=True,
                )
                nc.tensor.matmul(
                    ps3[0:1, B:2 * B], sel3_sb[:, 1:2], stats_sb[:],
                    start=True, stop=True,
                )
                zrec = workp.tile([1, B], f32, tag="zrec", name=f"zrec{t}")
                nc.vector.reciprocal(zrec[:], ps3[0:1, B:2 * B])
                zrow = workp.tile([1, B], f32, tag="zrow", name=f"zrow{t}")
                nc.vector.tensor_mul(zrow[:], ps3[0:1, 0:B], zrec[:])
                zcol = spsump.tile([B, 1], f32, tag="zcol", name=f"zcol{t}")
                nc.tensor.transpose(zcol[:], zrow[:], idn_sb[0:1, 0:1])
                nc.vector.tensor_copy(
                    zfull[:, t * LAT:(t + 1) * LAT],
                    zcol[:].to_broadcast([B, LAT]),
                )

            # ---------------- epilogue: write z ----------------------------
            nc.sync.dma_start(
                zout.ap().rearrange("b t l -> b (t l)"), zfull[:]
            )

    nc.compile()
    return nc


def _prep_inputs(input_tokens, E, Wi, Wh, b):
    """Host-side sharding / weight fusion. Returns per-core input maps."""
    EWi = (E.astype(np.float64) @ Wi.astype(np.float64)
           + b.astype(np.float64)).astype(np.float32)
    tok = np.ascontiguousarray(input_tokens.astype(np.int32))
    idn = np.eye(B, dtype=np.float32)
    # stats_sb partition p = r*4 + s  (r = rank, s = stat index, s=3 pad)
    sel3 = np.zeros((32, 2), np.float32)
    p = np.arange(32)
    sel3[(p % 4 == 1) | (p % 4 == 2), 0] = 1.5  # 1.5 * (num_lt + num_le)
    sel3[p % 4 == 0, 1] = 1.0                   # denom

    in_maps = []
    for k in range(NCORES):
        sl = np.arange(k * HS, (k + 1) * HS)
        cols = np.concatenate([sl, 2048 + sl, 6144 + sl, 4096 + sl])  # i,f,o,g
        in_maps.append({
            "wh": np.ascontiguousarray(Wh[:, cols], np.float32),
            "ewi": np.ascontiguousarray(EWi[:, cols], np.float32),
            "tok": tok,
            "iota": np.broadcast_to(
                (k * HS + np.arange(HS)).astype(np.float32)[None, :], (B, HS)
            ).copy(),
            "sel3": sel3,
            "idn": idn,
            "idnr": idn,
        })
    return in_maps


def kernel(input_tokens, E, Wi, Wh, b, _trace=False):
    from concourse import bass_utils

    if "nc" not in _CACHE:
        _CACHE["nc"] = _build_program()
    nc = _CACHE["nc"]

    in_maps = _prep_inputs(
        np.asarray(input_tokens), np.asarray(E), np.asarray(Wi),
        np.asarray(Wh), np.asarray(b),
    )
    res = bass_utils.run_bass_kernel_spmd(
        nc, in_maps, core_ids=list(range(NCORES)), trace=_trace,
    )
    _CACHE["last_result"] = res
    return np.asarray(res.results[0]["z"])

